# revision 11
# baseline (speedup 1.0000x reference)
"""Trainium2 Bass kernel for nn_AttentionBlock (sparse_attention).

Reference computation (fp32, single device):
    q = x @ WQ.T; k = x @ WK.T; v = x @ WV.T          # x: [8193, 1024]
    attn = (q @ k.T) * 0.03125
    attn[1:, 1:] += phi_spd + phi_edge + phi_3d        # phi: [8192, 8192]
    out = softmax(attn, -1) @ v                        # [8193, 1024]

Distribution (8 NeuronCores, SPMD, one identical program):
  - Q rows sharded: core c owns global row 0 (redundantly) plus real-atom
    rows [1 + c*1024, 1 + (c+1)*1024).  Row 0 of the output is taken from
    core 0.
  - K/V: each core projects k/v for its own 1025 rows, the 1024 real-atom
    columns are AllGather'ed in bf16; the row-0 column (k0/v0) is kept
    locally (identical on every core).
  - phi row-block is pre-summed/transposed on host and streamed per core.

Device kernel (per core), attnT layout ([j keys on partitions, i queries
on free dim]) so both matmuls and the softmax denominator run on the PE:
  qT = (WQ*s).T-proj, kT = WK-proj, v = WV-proj    (bf16 matmuls, fp32 psum)
  AllGather kT/v (bf16)
  for each i-block (512/512/1 query cols):
    pass A: for all 65 j-chunks: psum = kT.T @ qT (8 matmuls over d);
            psum += phiT tile (DVE); e = exp(psum) (ACT, bf16 out)
    pass B: for d-half: av[i_sub] += e.T @ v_half; den[i_sub] += e.T @ ones
    out = av * 1/den  (DVE), DMA out
Softmax max-subtraction is skipped: logits are bounded (|logit| < ~14),
exp stays well inside fp32/bf16 range, and softmax is shift-invariant.
"""

import numpy as np
import ml_dtypes

import concourse.bass as bass
import concourse.tile as tile
import concourse.mybir as mybir
from concourse.bass_utils import run_bass_kernel_spmd
from concourse.vector_clock import ScopedClock

BF16 = mybir.dt.bfloat16
F32 = mybir.dt.float32
AF = mybir.ActivationFunctionType
ALU = mybir.AluOpType

NCORES = 8
SCALING = 0.03125


# ---------------------------------------------------------------------------
# Workaround: this toolchain's walrus accepts at most one sem-wait on a Drain
# instruction, but TileContext._drain_and_barrier puts the whole global-clock
# wait set on a single drain.  Split the waits across a chain of drains.
def _patched_drain_and_barrier(self, tick_clock, wait_clock):
    nc = self.nc
    drain_inst = nc.sync.drain()
    wait_clock.add_sem_waits(
        drain_inst.ins, ScopedClock({None: tick_clock.global_clock})
    )
    si = drain_inst.ins.sync_info
    waits = list(si.on_wait) if si is not None and si.on_wait else []
    if len(waits) > 1:
        drain_inst.ins.sync_info = mybir.SyncInfo(
            on_wait=waits[:1], on_update=list(si.on_update or [])
        )
        for w in waits[1:]:
            extra = nc.sync.drain()
            extra.ins.sync_info = mybir.SyncInfo(on_wait=[w], on_update=[])
    nc.all_engine_barrier()
    assert self.sems is not None
    popped = nc._tile_sem_poison_stack.pop()
    assert popped is self._sem_poison
    nc.clear_and_free_semaphores(list(self.sems.allocated().values()))
    nc.all_engine_barrier()


tile.TileContext._drain_and_barrier = _patched_drain_and_barrier


def _split_multi_waits(nc):
    """Walrus here accepts at most one sem-wait per instruction.  Hoist extra
    waits onto same-engine NoOp carriers inserted just before the owner."""
    n = 0
    for fn in nc.m.functions:
        for bb in fn.blocks:
            out = []
            for inst in bb.instructions:
                si = inst.sync_info
                waits = list(si.on_wait) if si is not None and si.on_wait else []
                if len(waits) > 1:
                    for w in waits[:-1]:
                        nop = mybir.InstNoOp(
                            name=f"nopw-{n}", ins=[], outs=[],
                            engine=inst.engine)
                        n += 1
                        nop.sync_info = mybir.SyncInfo(on_wait=[w], on_update=[])
                        out.append(nop)
                    inst.sync_info = mybir.SyncInfo(
                        on_wait=[waits[-1]],
                        on_update=list(si.on_update or []))
                out.append(inst)
            bb.instructions = out
# ---------------------------------------------------------------------------


def build_nc(NA=8192, D=1024, debug=False, reps=1):
    """Build the SPMD Bass program (identical on all cores)."""
    SH = NA // NCORES          # atoms per core
    R = SH + 1                 # q rows per core (incl. global row 0)
    DC = D // 128              # 128-row chunks of the feature dim
    JCH = SH // 128            # j-chunks per core block
    JC = NA // 128             # real-atom j-chunks
    DH = D // 512              # 512-wide d halves of the output
    IB = []                    # i blocks (offset, width)
    off = 0
    while off < R:
        w = min(512, R - off)
        IB.append((off, w))
        off += w

    nc = bass.Bass(num_devices=NCORES)
    xT = nc.dram_tensor("xT", [D, R], BF16, kind="ExternalInput")
    wqT = nc.dram_tensor("wqT", [D, D], BF16, kind="ExternalInput")
    wkT = nc.dram_tensor("wkT", [D, D], BF16, kind="ExternalInput")
    wvT = nc.dram_tensor("wvT", [D, D], BF16, kind="ExternalInput")
    phiT = nc.dram_tensor("phiT", [NA, R], F32, kind="ExternalInput")
    out = nc.dram_tensor("out", [R, D], F32, kind="ExternalOutput")
    if debug:
        qT_d = nc.dram_tensor("qT_d", [128, DC, R], BF16, kind="ExternalOutput")
        kvg_d = nc.dram_tensor("kvg_d", [NCORES, 2, SH * D], BF16,
                               kind="ExternalOutput")
        logit_d = nc.dram_tensor("logit_d", [128, 512], F32, kind="ExternalOutput")
        e_d = nc.dram_tensor("e_d", [128, 512], BF16, kind="ExternalOutput")
        denr_d = nc.dram_tensor("denr_d", [128, 4], F32, kind="ExternalOutput")

    with tile.TileContext(nc) as tc:
        with tc.tile_pool(name="persist", bufs=1) as persist, \
             tc.tile_pool(name="epool", bufs=JC + 3) as epool, \
             tc.tile_pool(name="e0pool", bufs=3) as e0pool, \
             tc.tile_pool(name="kpool", bufs=2) as kpool, \
             tc.tile_pool(name="phipool", bufs=4) as phipool, \
             tc.tile_pool(name="vpool", bufs=4) as vpool, \
             tc.tile_pool(name="opool", bufs=4) as opool, \
             tc.tile_pool(name="spool", bufs=4) as spool, \
             tc.tile_pool(name="dram", bufs=1, space="DRAM") as dram:
          for _rep in range(reps):
            qT_sb = persist.tile([128, DC, R], BF16, tag="qT_sb", name="qT_sb")
            k0_sb = persist.tile([128, DC, 1], BF16, tag="k0_sb", name="k0_sb")
            v0_sb = persist.tile([1, D], BF16, tag="v0_sb", name="v0_sb")
            ones_sb = persist.tile([128, 1], BF16, tag="ones_sb", name="ones_sb")
            nc.vector.memset(ones_sb[:], 1.0)

            kv_in = dram.tile([2, SH * D], BF16, tag="kv_in", name="kv_in")
            kv_g = dram.tile([NCORES, 2, SH * D], BF16, addr_space="Shared",
                             tag="kv_g", name="kv_g")

            # ---------------- stage 1: projections ----------------
            with tc.tile_pool(name="projc", bufs=1) as projc, \
                 tc.tile_pool(name="wpool", bufs=1) as wpool, \
                 tc.tile_pool(name="pps", bufs=4, space="PSUM") as pps:
                xT_sb = projc.tile([128, DC, R], BF16)
                nc.sync.dma_start(
                    xT_sb[:], xT.rearrange("(c p) i -> p c i", p=128))

                islices = []
                o = 0
                while o < R:
                    islices.append((o, min(512, R - o)))
                    o += 512

                # qT[d, i]: accumulate over e-chunks; kept in SBUF
                wq_sb = wpool.tile([128, DC, D], BF16, tag="w", name="wq_sb")
                nc.sync.dma_start(
                    wq_sb[:], wqT.rearrange("(c p) i -> p c i", p=128))
                for dcol in range(DC):
                    for (i0, iw) in islices:
                        ps = pps.tile([128, 512], F32, tag="pps")
                        for ec in range(DC):
                            nc.tensor.matmul(
                                ps[:, :iw],
                                lhsT=wq_sb[:, ec, dcol * 128:(dcol + 1) * 128],
                                rhs=xT_sb[:, ec, i0:i0 + iw],
                                start=(ec == 0), stop=(ec == DC - 1))
                        nc.vector.tensor_copy(
                            qT_sb[:, dcol, i0:i0 + iw], ps[:, :iw])

                # kT[d, j]: drain straight to the allgather input (a = j-1);
                # column j=0 (global row 0) is kept locally in k0_sb
                wk_sb = wpool.tile([128, DC, D], BF16, tag="w", name="wk_sb")
                nc.sync.dma_start(
                    wk_sb[:], wkT.rearrange("(c p) i -> p c i", p=128))
                kvk = kv_in[0].rearrange("(c p a) -> p c a", p=128, c=DC)
                for dcol in range(DC):
                    for (i0, iw) in islices:
                        ps = pps.tile([128, 512], F32, tag="pps")
                        for ec in range(DC):
                            nc.tensor.matmul(
                                ps[:, :iw],
                                lhsT=wk_sb[:, ec, dcol * 128:(dcol + 1) * 128],
                                rhs=xT_sb[:, ec, i0:i0 + iw],
                                start=(ec == 0), stop=(ec == DC - 1))
                        kd = spool.tile([128, 512], BF16, tag="kdrain")
                        nc.vector.tensor_copy(kd[:, :iw], ps[:, :iw])
                        if i0 == 0:
                            nc.vector.tensor_copy(
                                k0_sb[:, dcol, 0:1], kd[:, 0:1])
                            nc.sync.dma_start(
                                kvk[:, dcol, 0:iw - 1], kd[:, 1:iw])
                        else:
                            nc.sync.dma_start(
                                kvk[:, dcol, i0 - 1:i0 - 1 + iw], kd[:, :iw])

                # v[a, d]: atoms only (rows 1..SH of the shard)
                wv_sb = wpool.tile([128, DC, D], BF16, tag="w", name="wv_sb")
                nc.sync.dma_start(
                    wv_sb[:], wvT.rearrange("(c p) i -> p c i", p=128))
                for ja in range(JCH):
                    for dh in range(DH):
                        ps = pps.tile([128, 512], F32, tag="pps")
                        for ec in range(DC):
                            nc.tensor.matmul(
                                ps[:],
                                lhsT=xT_sb[:, ec, 1 + ja * 128:1 + (ja + 1) * 128],
                                rhs=wv_sb[:, ec, dh * 512:(dh + 1) * 512],
                                start=(ec == 0), stop=(ec == DC - 1))
                        vtmp = spool.tile([128, 512], BF16, tag="vdrain")
                        nc.vector.tensor_copy(vtmp[:], ps[:])
                        nc.sync.dma_start(
                            kv_in[1].rearrange(
                                "(ja p d) -> p ja d", p=128, d=D)[
                                :, ja, dh * 512:(dh + 1) * 512],
                            vtmp[:])
                # v0 (global row 0)
                for dh in range(DH):
                    ps = pps.tile([1, 512], F32, tag="pps")
                    for ec in range(DC):
                        nc.tensor.matmul(
                            ps[:],
                            lhsT=xT_sb[:, ec, 0:1],
                            rhs=wv_sb[:, ec, dh * 512:(dh + 1) * 512],
                            start=(ec == 0), stop=(ec == DC - 1))
                    nc.vector.tensor_copy(v0_sb[:, dh * 512:(dh + 1) * 512], ps[:])

            # ---------------- stage 1.5: allgather K/V ----------------
            nc.gpsimd.collective_compute(
                "AllGather", ALU.bypass,
                replica_groups=[list(range(NCORES))],
                ins=[kv_in.opt()], outs=[kv_g.opt()])

            if debug:
                nc.sync.dma_start(qT_d[:], qT_sb[:])
                nc.sync.dma_start(kvg_d[:], kv_g[:])

            # ---------------- stage 2: attention ----------------
            with tc.tile_pool(name="aps", bufs=4, space="PSUM") as aps, \
                 tc.tile_pool(name="dps", bufs=1, space="PSUM") as dps:
                A_SPAN = min(512, SH)      # atoms per kT stream tile
                for (i0, w) in IB:
                    nsub = (w + 127) // 128
                    # ---- pass A: scores + exp, attnT layout [j, i]
                    e_tiles = []
                    for cb in range(NCORES):
                        for sp in range(SH // A_SPAN):
                          kt = kpool.tile([128, DC, A_SPAN], BF16, tag="kt")
                          nc.sync.dma_start(
                            kt[:],
                            kv_g[cb, 0].rearrange(
                                "(c p a) -> p c a", p=128, c=DC)[
                                :, :, sp * A_SPAN:(sp + 1) * A_SPAN])
                          for ja2 in range(A_SPAN // 128):
                            ja = sp * (A_SPAN // 128) + ja2
                            jc = cb * JCH + ja
                            ph = phipool.tile([128, 512], F32, tag="ph")
                            nc.sync.dma_start(
                                ph[:, :w],
                                phiT[jc * 128:(jc + 1) * 128, i0:i0 + w])
                            ps = aps.tile([128, 512], F32, tag="ps")
                            for dc_ in range(DC):
                                nc.tensor.matmul(
                                    ps[:, :w],
                                    lhsT=kt[:, dc_, ja2 * 128:(ja2 + 1) * 128],
                                    rhs=qT_sb[:, dc_, i0:i0 + w],
                                    start=(dc_ == 0), stop=(dc_ == DC - 1))
                            nc.vector.tensor_tensor(
                                ps[:, :w], ps[:, :w], ph[:, :w], ALU.add)
                            e = epool.tile([128, 512], BF16, tag="e")
                            nc.scalar.activation(e[:, :w], ps[:, :w], AF.Exp)
                            if debug and i0 == 0 and jc == 0:
                                lt = spool.tile([128, 512], F32, tag="lt")
                                nc.vector.tensor_copy(lt[:, :w], ps[:, :w])
                                nc.sync.dma_start(logit_d[:, :w], lt[:, :w])
                                nc.sync.dma_start(e_d[:, :w], e[:, :w])
                            e_tiles.append(e)
                    # virtual-atom column (j = global row 0), no phi
                    ps0 = aps.tile([1, 512], F32, tag="ps")
                    for dc_ in range(DC):
                        nc.tensor.matmul(
                            ps0[:, :w],
                            lhsT=k0_sb[:, dc_, :],
                            rhs=qT_sb[:, dc_, i0:i0 + w],
                            start=(dc_ == 0), stop=(dc_ == DC - 1))
                    e0 = e0pool.tile([1, 512], BF16, tag="e0")
                    nc.scalar.activation(e0[:, :w], ps0[:, :w], AF.Exp)

                    # ---- pass B: av = e.T @ v, den = e.T @ 1
                    dens = [dps.tile([128, 1], F32, tag=f"den{s}", name=f"den{s}")
                            for s in range(nsub)]
                    for half in range(DH):
                        avs = [aps.tile([128, 512], F32, tag="ps",
                                        name=f"av{s}")
                               for s in range(nsub)]
                        for cb in range(NCORES):
                            for ja in range(JCH):
                                jc = cb * JCH + ja
                                vt = vpool.tile([128, 512], BF16, tag="vt")
                                nc.sync.dma_start(
                                    vt[:],
                                    kv_g[cb, 1].rearrange(
                                        "(ja p d) -> p ja d", p=128, d=D)[
                                        :, ja, half * 512:(half + 1) * 512])
                                for s in range(nsub):
                                    sw = min(128, w - s * 128)
                                    eslice = e_tiles[jc][:, s * 128:s * 128 + sw]
                                    nc.tensor.matmul(
                                        avs[s][:sw, :],
                                        lhsT=eslice, rhs=vt[:],
                                        start=(jc == 0), stop=False)
                                    if half == 0:
                                        nc.tensor.matmul(
                                            dens[s][:sw, :],
                                            lhsT=eslice, rhs=ones_sb[:],
                                            start=(jc == 0), stop=False)
                        # virtual-atom contribution
                        for s in range(nsub):
                            sw = min(128, w - s * 128)
                            eslice = e0[:, s * 128:s * 128 + sw]
                            nc.tensor.matmul(
                                avs[s][:sw, :],
                                lhsT=eslice,
                                rhs=v0_sb[:, half * 512:(half + 1) * 512],
                                start=False, stop=True)
                            if half == 0:
                                nc.tensor.matmul(
                                    dens[s][:sw, :],
                                    lhsT=eslice, rhs=ones_sb[0:1, :],
                                    start=False, stop=True)
                        # drain: out = av / den
                        if half == 0:
                            denr = spool.tile([128, 4], F32, tag="denr")
                            for s in range(nsub):
                                sw = min(128, w - s * 128)
                                nc.vector.reciprocal(
                                    denr[:sw, s:s + 1], dens[s][:sw, :])
                            if debug and i0 == 0:
                                nc.sync.dma_start(denr_d[:], denr[:])
                        for s in range(nsub):
                            sw = min(128, w - s * 128)
                            ot = opool.tile([128, 512], F32, tag="o")
                            nc.vector.tensor_scalar(
                                ot[:sw, :], avs[s][:sw, :],
                                denr[:sw, s:s + 1], None, ALU.mult)
                            nc.sync.dma_start(
                                out[i0 + s * 128:i0 + s * 128 + sw,
                                    half * 512:(half + 1) * 512],
                                ot[:sw, :])
    _split_multi_waits(nc)
    return nc


_NC_CACHE = {}


def _get_nc(NA, D):
    key = (NA, D)
    if key not in _NC_CACHE:
        _NC_CACHE[key] = build_nc(NA, D)
    return _NC_CACHE[key]


def prep_inputs(x, phi_3d, phi_spd, phi_edge, WQ, WK, WV):
    """Host-side sharding: transposes, bf16 casts, phi row-blocks."""
    NA = phi_3d.shape[0]
    D = x.shape[1]
    SH = NA // NCORES
    R = SH + 1
    bf = ml_dtypes.bfloat16
    PHI = phi_3d + phi_spd + phi_edge
    xT = np.ascontiguousarray(np.asarray(x, dtype=np.float32).T)  # [D, NA+1]
    wqT = np.ascontiguousarray((np.asarray(WQ) * SCALING).T).astype(bf)
    wkT = np.ascontiguousarray(np.asarray(WK).T).astype(bf)
    wvT = np.ascontiguousarray(np.asarray(WV).T).astype(bf)
    in_maps = []
    for c in range(NCORES):
        xT_c = np.concatenate(
            [xT[:, 0:1], xT[:, 1 + c * SH:1 + (c + 1) * SH]], axis=1).astype(bf)
        phiT_c = np.zeros((NA, R), np.float32)
        phiT_c[:, 1:] = PHI[c * SH:(c + 1) * SH, :].T
        in_maps.append({"xT": xT_c, "wqT": wqT, "wkT": wkT, "wvT": wvT,
                        "phiT": phiT_c})
    return in_maps


def run(x, phi_3d, phi_spd, phi_edge, WQ, WK, WV, trace=False):
    NA = phi_3d.shape[0]
    D = x.shape[1]
    SH = NA // NCORES
    nc = _get_nc(NA, D)
    in_maps = prep_inputs(x, phi_3d, phi_spd, phi_edge, WQ, WK, WV)
    res = run_bass_kernel_spmd(nc, in_maps, list(range(NCORES)), trace=trace)
    full = np.empty((NA + 1, D), np.float32)
    full[0] = res.results[0]["out"][0]
    for c in range(NCORES):
        full[1 + c * SH:1 + (c + 1) * SH] = res.results[c]["out"][1:]
    return full, res


def kernel(x, phi_3d, phi_spd, phi_edge, delta_pos, WQ, WK, WV):
    out, _ = run(x, phi_3d, phi_spd, phi_edge, WQ, WK, WV)
    return out


# revision 15
# speedup vs baseline: 1.1828x; 1.1828x over previous
"""Trainium2 Bass kernel for nn_AttentionBlock (sparse_attention).

Reference computation (fp32, single device):
    q = x @ WQ.T; k = x @ WK.T; v = x @ WV.T          # x: [8193, 1024]
    attn = (q @ k.T) * 0.03125
    attn[1:, 1:] += phi_spd + phi_edge + phi_3d        # phi: [8192, 8192]
    out = softmax(attn, -1) @ v                        # [8193, 1024]

Distribution (8 NeuronCores, SPMD, one identical program):
  - Q rows sharded: core c owns global row 0 (redundantly) plus real-atom
    rows [1 + c*1024, 1 + (c+1)*1024).  Row 0 of the output is taken from
    core 0.
  - K/V: each core projects k/v for its own 1025 rows, the 1024 real-atom
    columns are AllGather'ed in bf16; the row-0 column (k0/v0) is kept
    locally (identical on every core).
  - phi row-block is pre-summed/transposed on host and streamed per core.

Device kernel (per core), attnT layout ([j keys on partitions, i queries
on free dim]) so both attention matmuls and the softmax denominator run on
the PE with no transposes:
  kT = WK-proj -> AllGather;  v = WV-proj -> AllGather;  qT = (WQ*s).T-proj
  (bf16 matmuls, fp32 psum; each collective overlaps the next projection)
  for each 512-wide i-block (the straggler query column i = R-1 rides along
  with the last block as "x2"/"02" pieces, reusing its kT/V stream):
    pass A: for all 64 j-chunks: psum = kT.T @ qT (8 matmuls over d);
            psum += phiT tile (DVE); e = exp(psum) (ACT, bf16 out)
            plus a j=row-0 virtual column from the locally kept k0
    pass B: for d-half: av[i_sub] += e.T @ v_half;
            den[:, i_sub] += e.T @ ones into one shared psum bank (half 0)
    out = av * recip(den)  (DVE), DMA out
Softmax max-subtraction is skipped: logits are bounded (|logit| < ~14),
exp stays well inside fp32/bf16 range, and softmax is shift-invariant.
"""

import numpy as np
import ml_dtypes

import concourse.bass as bass
import concourse.tile as tile
import concourse.mybir as mybir
from concourse.bass_utils import run_bass_kernel_spmd
from concourse.vector_clock import ScopedClock

BF16 = mybir.dt.bfloat16
F32 = mybir.dt.float32
AF = mybir.ActivationFunctionType
ALU = mybir.AluOpType

NCORES = 8
SCALING = 0.03125


# ---------------------------------------------------------------------------
# Workaround: this toolchain's walrus accepts at most one sem-wait on a Drain
# instruction, but TileContext._drain_and_barrier puts the whole global-clock
# wait set on a single drain.  Split the waits across a chain of drains.
def _patched_drain_and_barrier(self, tick_clock, wait_clock):
    nc = self.nc
    drain_inst = nc.sync.drain()
    wait_clock.add_sem_waits(
        drain_inst.ins, ScopedClock({None: tick_clock.global_clock})
    )
    si = drain_inst.ins.sync_info
    waits = list(si.on_wait) if si is not None and si.on_wait else []
    if len(waits) > 1:
        drain_inst.ins.sync_info = mybir.SyncInfo(
            on_wait=waits[:1], on_update=list(si.on_update or [])
        )
        for w in waits[1:]:
            extra = nc.sync.drain()
            extra.ins.sync_info = mybir.SyncInfo(on_wait=[w], on_update=[])
    nc.all_engine_barrier()
    assert self.sems is not None
    popped = nc._tile_sem_poison_stack.pop()
    assert popped is self._sem_poison
    nc.clear_and_free_semaphores(list(self.sems.allocated().values()))
    nc.all_engine_barrier()


tile.TileContext._drain_and_barrier = _patched_drain_and_barrier


def _split_multi_waits(nc):
    """Walrus here accepts at most one sem-wait per instruction.  Hoist extra
    waits onto same-engine NoOp carriers inserted just before the owner."""
    n = 0
    for fn in nc.m.functions:
        for bb in fn.blocks:
            out = []
            for inst in bb.instructions:
                si = inst.sync_info
                waits = list(si.on_wait) if si is not None and si.on_wait else []
                if len(waits) > 1:
                    for w in waits[:-1]:
                        nop = mybir.InstNoOp(
                            name=f"nopw-{n}", ins=[], outs=[],
                            engine=inst.engine)
                        n += 1
                        nop.sync_info = mybir.SyncInfo(on_wait=[w], on_update=[])
                        out.append(nop)
                    inst.sync_info = mybir.SyncInfo(
                        on_wait=[waits[-1]],
                        on_update=list(si.on_update or []))
                out.append(inst)
            bb.instructions = out
# ---------------------------------------------------------------------------


def build_nc(NA=8192, D=1024, debug=False, reps=1, tick=False):
    """Build the SPMD Bass program (identical on all cores)."""
    SH = NA // NCORES          # atoms per core
    R = SH + 1                 # q rows per core (incl. global row 0)
    DC = D // 128              # 128-row chunks of the feature dim
    JCH = SH // 128            # j-chunks per core block
    JC = NA // 128             # real-atom j-chunks
    DH = D // 512              # 512-wide d halves of the output
    A_SPAN = min(512, SH)      # atoms per kT stream tile
    # i blocks of <=512 query columns covering R-1 columns; the final
    # straggler column (i = R-1) rides along with the last block ("x2").
    n_strag = 1
    body_R = R - n_strag
    assert body_R % 128 == 0
    IB = []
    off = 0
    while off < body_R:
        wblk = min(512, body_R - off)
        IB.append((off, wblk))
        off += wblk

    nc = bass.Bass(num_devices=NCORES)
    xT = nc.dram_tensor("xT", [D, R], BF16, kind="ExternalInput")
    wqT = nc.dram_tensor("wqT", [D, D], BF16, kind="ExternalInput")
    wkT = nc.dram_tensor("wkT", [D, D], BF16, kind="ExternalInput")
    wvT = nc.dram_tensor("wvT", [D, D], BF16, kind="ExternalInput")
    phiT = nc.dram_tensor("phiT", [NA, R], F32, kind="ExternalInput")
    out = nc.dram_tensor("out", [R, D], F32, kind="ExternalOutput")
    tick_t = (nc.dram_tensor("tick", [1, 1], F32, kind="ExternalOutput")
              if tick else None)

    with tile.TileContext(nc) as tc:
        with tc.tile_pool(name="persist", bufs=1) as persist, \
             tc.tile_pool(name="epool", bufs=JC + 3) as epool, \
             tc.tile_pool(name="e2pool", bufs=JC + 3) as e2pool, \
             tc.tile_pool(name="e0pool", bufs=3) as e0pool, \
             tc.tile_pool(name="kpool", bufs=2) as kpool, \
             tc.tile_pool(name="phipool", bufs=4) as phipool, \
             tc.tile_pool(name="vpool", bufs=4) as vpool, \
             tc.tile_pool(name="opool", bufs=4) as opool, \
             tc.tile_pool(name="spool", bufs=4) as spool, \
             tc.tile_pool(name="dram", bufs=1, space="DRAM") as dram:
          for _rep in range(reps):
            qT_sb = persist.tile([128, DC, R], BF16, tag="qT_sb", name="qT_sb")
            k0_sb = persist.tile([128, DC, 1], BF16, tag="k0_sb", name="k0_sb")
            v0_sb = persist.tile([1, D], BF16, tag="v0_sb", name="v0_sb")
            ones_sb = persist.tile([128, 1], BF16, tag="ones_sb", name="ones_sb")
            nc.vector.memset(ones_sb[:], 1.0)
            zero8_sb = persist.tile([128, 8], BF16, tag="zero8_sb",
                                    name="zero8_sb")
            nc.vector.memset(zero8_sb[:], 0.0)

            k_in = dram.tile([D * SH], BF16, tag="k_in", name="k_in")
            k_g = dram.tile([NCORES, D * SH], BF16, addr_space="Shared",
                            tag="k_g", name="k_g")
            v_in = dram.tile([SH * D], BF16, tag="v_in", name="v_in")
            v_g = dram.tile([NCORES, SH * D], BF16, addr_space="Shared",
                            tag="v_g", name="v_g")

            # ---------------- stage 1: projections (K, V, then Q) --------
            islices = []
            o = 0
            while o < R:
                islices.append((o, min(512, R - o)))
                o += 512
            with tc.tile_pool(name="projc", bufs=1) as projc, \
                 tc.tile_pool(name="wpool", bufs=2) as wpool, \
                 tc.tile_pool(name="pps", bufs=4, space="PSUM") as pps:
                xT_sb = projc.tile([128, DC, R], BF16)
                xr = xT.rearrange("(c p) i -> p c i", p=128)
                for ec in range(DC):
                    nc.sync.dma_start(xT_sb[:, ec, :], xr[:, ec, :])

                # kT[d, j]: drain straight to the allgather input (a = j-1);
                # column j=0 (global row 0) is kept locally in k0_sb
                wk_sb = wpool.tile([128, DC, D], BF16, tag="w", name="wk_sb")
                wkr = wkT.rearrange("(c p) i -> p c i", p=128)
                for ec in range(DC):
                    nc.sync.dma_start(wk_sb[:, ec, :], wkr[:, ec, :])
                kvk = k_in.rearrange("(c p a) -> p c a", p=128, c=DC)
                for dcol in range(DC):
                    for (i0, iw) in islices:
                        ps = pps.tile([128, 512], F32, tag="pps")
                        for ec in range(DC):
                            nc.tensor.matmul(
                                ps[:, :iw],
                                lhsT=wk_sb[:, ec, dcol * 128:(dcol + 1) * 128],
                                rhs=xT_sb[:, ec, i0:i0 + iw],
                                start=(ec == 0), stop=(ec == DC - 1))
                        kd = spool.tile([128, 512], BF16, tag="kdrain")
                        nc.vector.tensor_copy(kd[:, :iw], ps[:, :iw])
                        if i0 == 0:
                            nc.vector.tensor_copy(
                                k0_sb[:, dcol, 0:1], kd[:, 0:1])
                            nc.sync.dma_start(
                                kvk[:, dcol, 0:iw - 1], kd[:, 1:iw])
                        else:
                            nc.sync.dma_start(
                                kvk[:, dcol, i0 - 1:i0 - 1 + iw], kd[:, :iw])
                nc.gpsimd.collective_compute(
                    "AllGather", ALU.bypass,
                    replica_groups=[list(range(NCORES))],
                    ins=[k_in.opt()], outs=[k_g.opt()])

                # v[a, d]: atoms only (rows 1..SH of the shard)
                wv_sb = wpool.tile([128, DC, D], BF16, tag="w", name="wv_sb")
                wvr = wvT.rearrange("(c p) i -> p c i", p=128)
                for ec in range(DC):
                    nc.sync.dma_start(wv_sb[:, ec, :], wvr[:, ec, :])
                for ja in range(JCH):
                    for dh in range(DH):
                        ps = pps.tile([128, 512], F32, tag="pps")
                        for ec in range(DC):
                            nc.tensor.matmul(
                                ps[:],
                                lhsT=xT_sb[:, ec, 1 + ja * 128:1 + (ja + 1) * 128],
                                rhs=wv_sb[:, ec, dh * 512:(dh + 1) * 512],
                                start=(ec == 0), stop=(ec == DC - 1))
                        vtmp = spool.tile([128, 512], BF16, tag="vdrain")
                        nc.vector.tensor_copy(vtmp[:], ps[:])
                        nc.sync.dma_start(
                            v_in.rearrange(
                                "(ja p d) -> p ja d", p=128, d=D)[
                                :, ja, dh * 512:(dh + 1) * 512],
                            vtmp[:])
                # v0 (global row 0)
                for dh in range(DH):
                    ps = pps.tile([1, 512], F32, tag="pps")
                    for ec in range(DC):
                        nc.tensor.matmul(
                            ps[:],
                            lhsT=xT_sb[:, ec, 0:1],
                            rhs=wv_sb[:, ec, dh * 512:(dh + 1) * 512],
                            start=(ec == 0), stop=(ec == DC - 1))
                    nc.vector.tensor_copy(v0_sb[:, dh * 512:(dh + 1) * 512], ps[:])
                nc.gpsimd.collective_compute(
                    "AllGather", ALU.bypass,
                    replica_groups=[list(range(NCORES))],
                    ins=[v_in.opt()], outs=[v_g.opt()])

                # qT[d, i]: accumulate over e-chunks; kept in SBUF
                wq_sb = wpool.tile([128, DC, D], BF16, tag="w", name="wq_sb")
                wqr = wqT.rearrange("(c p) i -> p c i", p=128)
                for ec in range(DC):
                    nc.sync.dma_start(wq_sb[:, ec, :], wqr[:, ec, :])
                for dcol in range(DC):
                    for (i0, iw) in islices:
                        ps = pps.tile([128, 512], F32, tag="pps")
                        for ec in range(DC):
                            nc.tensor.matmul(
                                ps[:, :iw],
                                lhsT=wq_sb[:, ec, dcol * 128:(dcol + 1) * 128],
                                rhs=xT_sb[:, ec, i0:i0 + iw],
                                start=(ec == 0), stop=(ec == DC - 1))
                        nc.vector.tensor_copy(
                            qT_sb[:, dcol, i0:i0 + iw], ps[:, :iw])

            # ---------------- stage 2: attention ----------------
            # psum: aps 6 banks (scores pipeline / 4 av accumulators + av2)
            #       dps 2 banks (den columns, one bank per i-block)
            with tc.tile_pool(name="aps", bufs=6, space="PSUM") as aps, \
                 tc.tile_pool(name="dps", bufs=2, space="PSUM") as dps:
                kgr = k_g.rearrange("n (c p a) -> n p c a", p=128, c=DC)
                vgr = v_g.rearrange("n (ja p d) -> n p ja d", p=128, d=D)
                for ib, (i0, w) in enumerate(IB):
                    last = (ib == len(IB) - 1)
                    x2 = bool(n_strag) and last      # extra query column i = R-1
                    nsub = w // 128
                    # ---- pass A: scores + exp, attnT layout [j, i]
                    e_tiles, e2_tiles = [], []
                    for cb in range(NCORES):
                        for sp in range(SH // A_SPAN):
                          kt = kpool.tile([128, DC, A_SPAN], BF16, tag="kt")
                          nc.sync.dma_start(
                            kt[:],
                            kgr[cb][:, :, sp * A_SPAN:(sp + 1) * A_SPAN])
                          for ja2 in range(A_SPAN // 128):
                            ja = sp * (A_SPAN // 128) + ja2
                            jc = cb * JCH + ja
                            phw = w + (1 if x2 else 0)
                            ph = phipool.tile([128, 520], F32, tag="ph")
                            nc.sync.dma_start(
                                ph[:, :phw],
                                phiT[jc * 128:(jc + 1) * 128, i0:i0 + phw])
                            ps = aps.tile([128, 512], F32, tag="ps")
                            for dc_ in range(DC):
                                nc.tensor.matmul(
                                    ps[:, :w],
                                    lhsT=kt[:, dc_, ja2 * 128:(ja2 + 1) * 128],
                                    rhs=qT_sb[:, dc_, i0:i0 + w],
                                    start=(dc_ == 0), stop=(dc_ == DC - 1))
                            nc.vector.tensor_tensor(
                                ps[:, :w], ps[:, :w], ph[:, :w], ALU.add)
                            e = epool.tile([128, 512], BF16, tag="e")
                            nc.scalar.activation(e[:, :w], ps[:, :w], AF.Exp)
                            e_tiles.append(e)
                            if x2:
                                ps2 = aps.tile([128, 1], F32, tag="ps",
                                               name="ps2")
                                for dc_ in range(DC):
                                    nc.tensor.matmul(
                                        ps2[:],
                                        lhsT=kt[:, dc_,
                                                ja2 * 128:(ja2 + 1) * 128],
                                        rhs=qT_sb[:, dc_, R - 1:R],
                                        start=(dc_ == 0), stop=(dc_ == DC - 1))
                                nc.vector.tensor_tensor(
                                    ps2[:], ps2[:], ph[:, w:w + 1], ALU.add)
                                e2 = e2pool.tile([128, 1], BF16, tag="e2")
                                nc.scalar.activation(e2[:], ps2[:], AF.Exp)
                                e2_tiles.append(e2)
                    # virtual-atom column (j = global row 0), no phi
                    ps0 = aps.tile([1, 512], F32, tag="ps", name="ps0")
                    for dc_ in range(DC):
                        nc.tensor.matmul(
                            ps0[:, :w],
                            lhsT=k0_sb[:, dc_, :],
                            rhs=qT_sb[:, dc_, i0:i0 + w],
                            start=(dc_ == 0), stop=(dc_ == DC - 1))
                    e0 = e0pool.tile([1, 512], BF16, tag="e0")
                    nc.scalar.activation(e0[:, :w], ps0[:, :w], AF.Exp)
                    if x2:
                        ps02 = aps.tile([1, 1], F32, tag="ps", name="ps02")
                        for dc_ in range(DC):
                            nc.tensor.matmul(
                                ps02[:],
                                lhsT=k0_sb[:, dc_, :],
                                rhs=qT_sb[:, dc_, R - 1:R],
                                start=(dc_ == 0), stop=(dc_ == DC - 1))
                        e02 = e0pool.tile([1, 1], BF16, tag="e02", name="e02")
                        nc.scalar.activation(e02[:], ps02[:], AF.Exp)

                    # ---- pass B: av = e.T @ v; den[i_sub, s] accumulated in
                    # one psum bank: a full-coverage start=True matmul against
                    # a zero rhs sets has_written for the whole region, then
                    # every den group accumulates with start=False (psum bank
                    # sharing with start=True per group corrupts neighbors).
                    den = dps.tile([128, 8], F32, tag="den", name="den")
                    nc.tensor.matmul(
                        den[:], lhsT=e_tiles[0][:, 0:128], rhs=zero8_sb[:],
                        start=True, stop=False, skip_group_check=True)
                    for half in range(DH):
                        avs = [aps.tile([128, 512], F32, tag="ps",
                                        name=f"av{s}")
                               for s in range(nsub)]
                        if x2:
                            av2 = aps.tile([1, 512], F32, tag="ps", name="av2")
                        for cb in range(NCORES):
                            for ja in range(JCH):
                                jc = cb * JCH + ja
                                vt = vpool.tile([128, 512], BF16, tag="vt")
                                nc.sync.dma_start(
                                    vt[:],
                                    vgr[cb][:, ja, half * 512:(half + 1) * 512])
                                for s in range(nsub):
                                    nc.tensor.matmul(
                                        avs[s][:],
                                        lhsT=e_tiles[jc][:, s * 128:(s + 1) * 128],
                                        rhs=vt[:],
                                        start=(jc == 0), stop=False)
                                if x2:
                                    nc.tensor.matmul(
                                        av2[:], lhsT=e2_tiles[jc], rhs=vt[:],
                                        start=(jc == 0), stop=False)
                                if half == 0:
                                    for s in range(nsub):
                                        nc.tensor.matmul(
                                            den[0:128, s:s + 1],
                                            lhsT=e_tiles[jc][:, s * 128:(s + 1) * 128],
                                            rhs=ones_sb[:],
                                            start=False, stop=False,
                                            skip_group_check=True)
                                    if x2:
                                        nc.tensor.matmul(
                                            den[0:1, 4:5],
                                            lhsT=e2_tiles[jc], rhs=ones_sb[:],
                                            start=False, stop=False,
                                            skip_group_check=True)
                        # virtual-atom contribution (K=1)
                        for s in range(nsub):
                            nc.tensor.matmul(
                                avs[s][:],
                                lhsT=e0[:, s * 128:(s + 1) * 128],
                                rhs=v0_sb[:, half * 512:(half + 1) * 512],
                                start=False, stop=True)
                        if x2:
                            nc.tensor.matmul(
                                av2[:], lhsT=e02[:],
                                rhs=v0_sb[:, half * 512:(half + 1) * 512],
                                start=False, stop=True)
                        if half == 0:
                            for s in range(nsub):
                                nc.tensor.matmul(
                                    den[0:128, s:s + 1],
                                    lhsT=e0[:, s * 128:(s + 1) * 128],
                                    rhs=ones_sb[0:1, :],
                                    start=False, stop=False,
                                    skip_group_check=True)
                            if x2:
                                nc.tensor.matmul(
                                    den[0:1, 4:5],
                                    lhsT=e02[:], rhs=ones_sb[0:1, :],
                                    start=False, stop=True,
                                    skip_group_check=True)
                            denr = spool.tile([128, 8], F32, tag="denr",
                                              name="denr")
                            nc.vector.reciprocal(denr[:], den[:])
                        for s in range(nsub):
                            ot = opool.tile([128, 512], F32, tag="o")
                            nc.vector.tensor_scalar(
                                ot[:], avs[s][:],
                                denr[:, s:s + 1], None, ALU.mult)
                            nc.sync.dma_start(
                                out[i0 + s * 128:i0 + (s + 1) * 128,
                                    half * 512:(half + 1) * 512],
                                ot[:])
                        if x2:
                            ot2 = opool.tile([1, 512], F32, tag="o2",
                                             name="ot2")
                            nc.vector.tensor_scalar(
                                ot2[:], av2[:], denr[0:1, 4:5], None, ALU.mult)
                            nc.sync.dma_start(
                                out[R - 1:R, half * 512:(half + 1) * 512],
                                ot2[:])
    if tick is not False:
        with tile.TileContext(nc) as tc2:
            with tc2.tile_pool(name="tickp", bufs=1) as tp:
                tk = tp.tile([1, 1], F32)
                nc.vector.memset(tk[:], 1.0)
                nc.sync.dma_start(tick_t[:], tk[:])
    _split_multi_waits(nc)
    return nc


_NC_CACHE = {}


def _get_nc(NA, D):
    key = (NA, D)
    if key not in _NC_CACHE:
        _NC_CACHE[key] = build_nc(NA, D)
    return _NC_CACHE[key]


def prep_inputs(x, phi_3d, phi_spd, phi_edge, WQ, WK, WV):
    """Host-side sharding: transposes, bf16 casts, phi row-blocks."""
    NA = phi_3d.shape[0]
    D = x.shape[1]
    SH = NA // NCORES
    R = SH + 1
    bf = ml_dtypes.bfloat16
    PHI = phi_3d + phi_spd + phi_edge
    xT = np.ascontiguousarray(np.asarray(x, dtype=np.float32).T)  # [D, NA+1]
    wqT = np.ascontiguousarray((np.asarray(WQ) * SCALING).T).astype(bf)
    wkT = np.ascontiguousarray(np.asarray(WK).T).astype(bf)
    wvT = np.ascontiguousarray(np.asarray(WV).T).astype(bf)
    in_maps = []
    for c in range(NCORES):
        xT_c = np.concatenate(
            [xT[:, 0:1], xT[:, 1 + c * SH:1 + (c + 1) * SH]], axis=1).astype(bf)
        phiT_c = np.zeros((NA, R), np.float32)
        phiT_c[:, 1:] = PHI[c * SH:(c + 1) * SH, :].T
        in_maps.append({"xT": xT_c, "wqT": wqT, "wkT": wkT, "wvT": wvT,
                        "phiT": phiT_c})
    return in_maps


def run(x, phi_3d, phi_spd, phi_edge, WQ, WK, WV, trace=False):
    NA = phi_3d.shape[0]
    D = x.shape[1]
    SH = NA // NCORES
    nc = _get_nc(NA, D)
    in_maps = prep_inputs(x, phi_3d, phi_spd, phi_edge, WQ, WK, WV)
    res = run_bass_kernel_spmd(nc, in_maps, list(range(NCORES)), trace=trace)
    full = np.empty((NA + 1, D), np.float32)
    full[0] = res.results[0]["out"][0]
    for c in range(NCORES):
        full[1 + c * SH:1 + (c + 1) * SH] = res.results[c]["out"][1:]
    return full, res


def kernel(x, phi_3d, phi_spd, phi_edge, delta_pos, WQ, WK, WV):
    out, _ = run(x, phi_3d, phi_spd, phi_edge, WQ, WK, WV)
    return out

